# revision 1
# baseline (speedup 1.0000x reference)
"""Location-sensitive attention TRN2 Bass kernel.

Data-parallel over batch: B=64 sharded as 8 per NeuronCore across 8 cores;
parameters replicated. Per core:

  query   = decoder_hidden @ Wq                     [8, 128]   (prep, on PE)
  keys    = encoder_outputs @ Wk                    [8, 2048, 128]
  loc     = conv1d(prev_attention) ; loc_term = loc @ Wl
  energy  = tanh(keys + query + loc_term) @ v       [8, 2048]
  out     = softmax(energy, axis=T)

Design notes (measured on HW):
 * enc arrives [tok, feat] with feat contiguous; the PE matmul contracts over
   the partition dim, so enc is transposed on-chip. The transposes are issued
   as REGULAR fp16 matmuls against an identity (out = lhsT.T @ I), not
   transpose-mode ops: transpose-mode does not register as PE activity in the
   HAM clock monitor, which leaves the whole PE stream throttled at 1.2 GHz
   (measured 4.8x slower end-to-end).
 * fp16 operands: 1 cyc/row matmul rate (4x faster than fp32, same as bf16)
   with a 10-bit mantissa (~1e-3 rel err vs ~1e-2 for bf16).
 * enc is loaded in 2MB-read chunks (1024 tokens), partition p holding 8
   consecutive tokens (8KB contiguous HBM per partition): measured 422 GB/s
   vs 217 GB/s for the partition-strided layout. Token order inside a group
   becomes t = 8p + s; softmax sums are order-invariant and the final
   normalization multiply un-permutes via its read access pattern.
 * enc loads ride SWDGE (gpsimd; f32->fp16 cast in the DMA). Small DMAs
   (exp-row assembly, output) ride the sync HWDGE ring so neither queue
   head-of-line blocks the other.
 * conv+Wl+conv_b fold into one [32, 128] matrix WW (rows 0..30 =
   sum_c conv_w[c,k]*Wl[c,:], row 31 = conv_b @ Wl), applied against a
   [32, 2048] shifted-window view of prev_attention (row 31 = ones) and
   accumulated into the same PSUM tile as the keys matmuls. query is the
   per-partition bias of the tanh activation. exp needs no max-subtraction:
   |energy| <= ||v||_1 ~ 11, safely inside fp32 exp range.
"""
import sys

sys.path.insert(0, "/opt/trn_rl_repo")

from contextlib import ExitStack

import numpy as np

import concourse.bass as bass
import concourse.tile as tile
from concourse import bacc, mybir
from concourse.bass_utils import run_bass_kernel_spmd
from concourse.masks import make_identity

B, T, ENC_DIM = 64, 2048, 512
Q_DIM, ATTN, CH, KS, PAD = 256, 128, 32, 31, 15
N_CORES = 8
BL = B // N_CORES  # 8 batches per core

f32 = mybir.dt.float32
fp16 = mybir.dt.float16
AF = mybir.ActivationFunctionType


def build(reps: int = 1):
    nc = bacc.Bacc("TRN2", target_bir_lowering=False, debug=False,
                   num_devices=N_CORES)

    enc_d = nc.dram_tensor("encoder_outputs", [BL, T, ENC_DIM], f32,
                           kind="ExternalInput").ap()
    dh_d = nc.dram_tensor("decoder_hidden", [BL, Q_DIM], f32,
                          kind="ExternalInput").ap()
    pa_d = nc.dram_tensor("prev_attention", [BL, T], f32,
                          kind="ExternalInput").ap()
    wq_d = nc.dram_tensor("Wq", [Q_DIM, ATTN], f32, kind="ExternalInput").ap()
    wk_d = nc.dram_tensor("Wk", [ENC_DIM, ATTN], f32, kind="ExternalInput").ap()
    cw_d = nc.dram_tensor("conv_w", [CH, 1, KS], f32, kind="ExternalInput").ap()
    cb_d = nc.dram_tensor("conv_b", [CH], f32, kind="ExternalInput").ap()
    wl_d = nc.dram_tensor("Wl", [CH, ATTN], f32, kind="ExternalInput").ap()
    v_d = nc.dram_tensor("v", [ATTN], f32, kind="ExternalInput").ap()
    out_d = nc.dram_tensor("out", [BL, T], f32, kind="ExternalOutput").ap()

    # internal DRAM scratch for the zero-padded prev_attention rows
    pa_pad_d = nc.dram_tensor("pa_pad", [BL, T + 32], f32).ap()

    with tile.TileContext(nc) as tc, ExitStack() as ctx:
        singles = ctx.enter_context(tc.tile_pool(name="singles", bufs=1))
        sb_enc = ctx.enter_context(tc.tile_pool(name="enc", bufs=4))
        sb_xt = ctx.enter_context(tc.tile_pool(name="xt", bufs=4))
        sb_tanh = ctx.enter_context(tc.tile_pool(name="tanh", bufs=3))
        sb_sm = ctx.enter_context(tc.tile_pool(name="sm", bufs=2))
        ps_xt = ctx.enter_context(tc.tile_pool(name="ps_xt", bufs=3, space="PSUM"))
        ps_o = ctx.enter_context(tc.tile_pool(name="ps_o", bufs=4, space="PSUM"))
        ps_prep = ctx.enter_context(tc.tile_pool(name="ps_prep", bufs=1, space="PSUM"))

        # ---------------- constants ----------------
        ident_f = singles.tile([128, 128], f32)
        make_identity(nc, ident_f)
        ident = singles.tile([128, 128], fp16)
        nc.vector.tensor_copy(ident, ident_f)

        wk_sb = singles.tile([128, 4, ATTN], fp16)
        nc.gpsimd.dma_start(wk_sb, wk_d.rearrange("(c k) a -> k c a", c=4))
        wq_sb = singles.tile([128, 2, ATTN], fp16)
        nc.gpsimd.dma_start(wq_sb, wq_d.rearrange("(c k) a -> k c a", c=2))
        dh_sb = singles.tile([BL, Q_DIM], fp16)
        nc.gpsimd.dma_start(dh_sb, dh_d)
        cwb_sb = singles.tile([CH, 32], fp16)
        nc.gpsimd.dma_start(cwb_sb[:, 0:KS], cw_d.rearrange("c o k -> c (o k)"))
        nc.gpsimd.dma_start(
            cwb_sb[:, KS:KS + 1],
            bass.AP(tensor=cb_d.tensor, offset=0, ap=[[1, CH], [1, 1]]))
        wl_sb = singles.tile([CH, ATTN], fp16)
        nc.gpsimd.dma_start(wl_sb, wl_d)
        v_sb = singles.tile([ATTN, 1], fp16)
        nc.gpsimd.dma_start(
            v_sb, bass.AP(tensor=v_d.tensor, offset=0, ap=[[1, ATTN], [1, 1]]))

        # ---------------- prep: queryT, WW ----------------
        # dhT [256, 8] via two transpose-matmuls of dh [8, 256]
        dhT_ps = ps_prep.tile([128, 2, BL], f32, tag="prep")
        for c in range(2):
            nc.tensor.matmul(dhT_ps[:, c, :], dh_sb[:, c * 128:(c + 1) * 128],
                             ident[0:BL, 0:BL], start=True, stop=True)
        dhT_sb = singles.tile([128, 2, BL], fp16)
        nc.vector.tensor_copy(dhT_sb, dhT_ps)

        # queryT [A, 8] = Wq.T @ dhT  (accumulate 2 chunks of q-dim)
        qt_ps = ps_prep.tile([ATTN, BL], f32, tag="prep")
        for c in range(2):
            nc.tensor.matmul(qt_ps, wq_sb[:, c, :], dhT_sb[:, c, :],
                             start=(c == 0), stop=(c == 1))
        qt_sb = singles.tile([ATTN, BL], f32)
        nc.scalar.copy(qt_sb, qt_ps)

        # WW [32, A]: rows 0..30 = sum_c conv_w[c,k] Wl[c,:], row 31 = conv_b @ Wl
        ww_ps = ps_prep.tile([32, ATTN], f32, tag="prep")
        nc.tensor.matmul(ww_ps, cwb_sb, wl_sb, start=True, stop=True)
        ww_sb = singles.tile([32, ATTN], fp16)
        nc.vector.tensor_copy(ww_sb, ww_ps)

        # ---------------- prep: shifted prev_attention windows ----------------
        pa_stage = singles.tile([BL, T + 32], f32)
        nc.vector.memset(pa_stage, 0.0)
        nc.sync.dma_start(pa_stage[:, PAD:PAD + T], pa_d)
        nc.sync.dma_start(pa_pad_d, pa_stage)

        ones_sb = singles.tile([1, T], f32)
        nc.vector.memset(ones_sb, 1.0)
        ones_d = nc.dram_tensor("ones_row", [T], f32).ap()
        nc.sync.dma_start(ones_d, ones_sb)

        # pa_sh[k, b, t] = pa_pad[b, t + k]  (k=0..30), row 31 = ones
        pa_sh = singles.tile([32, BL, T], fp16)
        nc.gpsimd.dma_start(
            pa_sh[0:KS, :, :],
            bass.AP(tensor=pa_pad_d.tensor, offset=0,
                    ap=[[1, KS], [T + 32, BL], [1, T]]))
        nc.gpsimd.dma_start(
            pa_sh[KS:KS + 1, :, :],
            bass.AP(tensor=ones_d.tensor, offset=0,
                    ap=[[0, 1], [0, BL], [1, T]]))

        # ---------------- main loop ----------------
        # supertile = 1024 tokens; partition p holds tokens 8p..8p+7 of it.
        NSUP = T // 1024  # 2 per batch row
        for rep in range(reps):
            exp_sb = sb_sm.tile([BL, T], f32, tag="exp")
            for b in range(BL):
                for G in range(NSUP):
                    enc_sb = sb_enc.tile([128, 8, ENC_DIM], fp16, tag="enc")
                    nc.gpsimd.dma_start(
                        enc_sb,
                        enc_d[b, G * 1024:(G + 1) * 1024, :]
                        .rearrange("(p s) f -> p s f", p=128))
                    for h in range(2):
                        out_ps = ps_o.tile([ATTN, 512], f32, tag="o")
                        for c in range(4):
                            xt_ps = ps_xt.tile([128, 512], f32, tag="xt")
                            for q in range(4):
                                nc.tensor.matmul(
                                    xt_ps[:, q * 128:(q + 1) * 128],
                                    enc_sb[:, 4 * h + q, c * 128:(c + 1) * 128],
                                    ident, start=True, stop=True)
                            xt_sb = sb_xt.tile([128, 512], fp16, tag="xts")
                            if c == 3:
                                nc.scalar.copy(xt_sb, xt_ps)
                            else:
                                nc.vector.tensor_copy(xt_sb, xt_ps)
                            nc.tensor.matmul(out_ps, wk_sb[:, c, :], xt_sb,
                                             start=(c == 0), stop=False)
                        # loc term: pa columns in permuted token order
                        # col j=(q,p) -> token 8p + 4h + q of this supertile
                        _sl = pa_sh[:, b, G * 1024 + 4 * h:]
                        pa_slice = bass.AP(tensor=_sl.tensor, offset=_sl.offset,
                                           ap=[_sl.ap[0], [1, 4], [8, 128]])
                        nc.tensor.matmul(out_ps, ww_sb, pa_slice,
                                         start=False, stop=True)

                        tanh_sb = sb_tanh.tile([ATTN, 512], fp16, tag="tanh")
                        nc.scalar.activation(tanh_sb, out_ps, AF.Tanh,
                                             bias=qt_sb[:, b:b + 1])

                        e_ps = ps_o.tile([1, 512], f32, tag="o")
                        nc.tensor.matmul(e_ps, v_sb, tanh_sb, start=True,
                                         stop=True)
                        exp_g = sb_xt.tile([1, 512], f32, tag="expg")
                        nc.scalar.activation(exp_g, e_ps, AF.Exp)
                        # ACT cannot write at partition base b; HWDGE DMA can
                        # (sync ring: keeps it off the SWDGE enc stream).
                        nc.sync.dma_start(
                            exp_sb[b:b + 1,
                                   G * 1024 + h * 512:G * 1024 + (h + 1) * 512],
                            exp_g)

            # softmax normalization over T, batched across the 8 rows
            sums = sb_sm.tile([BL, 1], f32, tag="sums")
            nc.vector.reduce_sum(sums, exp_sb, axis=mybir.AxisListType.X)
            inv = sb_sm.tile([BL, 1], f32, tag="inv")
            nc.vector.reciprocal(inv, sums)
            o_sb = sb_sm.tile([BL, T], f32, tag="osb")
            # un-permute: natural token t = G*1024 + 8p + 4h + q reads storage
            # index G*1024 + h*512 + q*128 + p
            _e = exp_sb[:, :]
            exp_perm = bass.AP(
                tensor=_e.tensor, offset=_e.offset,
                ap=[_e.ap[0], [1024, NSUP], [1, 128], [512, 2], [128, 4]])
            nc.vector.tensor_scalar_mul(o_sb, exp_perm, inv)
            nc.sync.dma_start(out_d, o_sb)

    nc.compile()
    return nc


_cache = {}


def _get(reps: int = 1):
    if reps not in _cache:
        _cache[reps] = build(reps)
    return _cache[reps]


def _in_maps(inputs):
    enc = np.ascontiguousarray(np.asarray(inputs["encoder_outputs"], dtype=np.float32))
    dh = np.ascontiguousarray(np.asarray(inputs["decoder_hidden"], dtype=np.float32))
    pa = np.ascontiguousarray(np.asarray(inputs["prev_attention"], dtype=np.float32))
    rep = {k: np.ascontiguousarray(np.asarray(inputs[k], dtype=np.float32))
           for k in ("Wq", "Wk", "conv_w", "conv_b", "Wl", "v")}
    maps = []
    for i in range(N_CORES):
        s = slice(i * BL, (i + 1) * BL)
        maps.append({"encoder_outputs": enc[s], "decoder_hidden": dh[s],
                     "prev_attention": pa[s], **rep})
    return maps


def kernel(**inputs) -> np.ndarray:
    nc = _get(1)
    res = run_bass_kernel_spmd(nc, _in_maps(inputs), list(range(N_CORES)))
    return np.concatenate([res.results[i]["out"] for i in range(N_CORES)],
                          axis=0).astype(np.float32)


if __name__ == "__main__":
    rng = np.random.default_rng(0)
    ins = {
        "encoder_outputs": rng.standard_normal((B, T, ENC_DIM), dtype=np.float32),
        "decoder_hidden": rng.standard_normal((B, Q_DIM), dtype=np.float32),
        "prev_attention": rng.random((B, T), dtype=np.float32),
        "Wq": (rng.standard_normal((Q_DIM, ATTN), dtype=np.float32) / np.sqrt(Q_DIM)),
        "Wk": (rng.standard_normal((ENC_DIM, ATTN), dtype=np.float32) / np.sqrt(ENC_DIM)),
        "conv_w": (rng.standard_normal((CH, 1, KS), dtype=np.float32) / np.sqrt(KS)),
        "conv_b": np.zeros(CH, dtype=np.float32),
        "Wl": (rng.standard_normal((CH, ATTN), dtype=np.float32) / np.sqrt(CH)),
        "v": (rng.standard_normal(ATTN, dtype=np.float32) / np.sqrt(ATTN)),
    }
    out = kernel(**ins)
    print("kernel output", out.shape, out.dtype, "row sums ~1:",
          np.allclose(out.sum(axis=1), 1.0, atol=1e-3))



# revision 2
# speedup vs baseline: 1.0603x; 1.0603x over previous
"""Location-sensitive attention TRN2 Bass kernel.

Data-parallel over batch: B=64 sharded as 8 per NeuronCore across 8 cores;
parameters replicated. Per core:

  query   = decoder_hidden @ Wq                     [8, 128]   (prep, on PE)
  keys    = encoder_outputs @ Wk                    [8, 2048, 128]
  loc     = conv1d(prev_attention) ; loc_term = loc @ Wl
  energy  = tanh(keys + query + loc_term) @ v       [8, 2048]
  out     = softmax(energy, axis=T)

Design notes (measured on HW):
 * enc arrives [tok, feat] with feat contiguous; the PE matmul contracts over
   the partition dim, so enc is transposed on-chip. The transposes are issued
   as REGULAR fp16 matmuls against an identity (out = lhsT.T @ I), not
   transpose-mode ops: transpose-mode does not register as PE activity in the
   HAM clock monitor, which leaves the whole PE stream throttled at 1.2 GHz
   (measured 4.8x slower end-to-end).
 * fp16 operands: 1 cyc/row matmul rate (4x faster than fp32, same as bf16)
   with a 10-bit mantissa (~1e-3 rel err vs ~1e-2 for bf16).
 * enc is loaded in 2MB-read chunks (1024 tokens), partition p holding 8
   consecutive tokens (8KB contiguous HBM per partition): measured 422 GB/s
   vs 217 GB/s for the partition-strided layout. Token order inside a group
   becomes t = 8p + s; softmax sums are order-invariant and the final
   normalization multiply un-permutes via its read access pattern.
 * enc loads ride SWDGE (gpsimd; f32->fp16 cast in the DMA). Small DMAs
   (exp-row assembly, output) ride the sync HWDGE ring so neither queue
   head-of-line blocks the other.
 * conv+Wl+conv_b fold into one [32, 128] matrix WW (rows 0..30 =
   sum_c conv_w[c,k]*Wl[c,:], row 31 = conv_b @ Wl), applied against a
   [32, 2048] shifted-window view of prev_attention (row 31 = ones) and
   accumulated into the same PSUM tile as the keys matmuls. query is the
   per-partition bias of the tanh activation. exp needs no max-subtraction:
   |energy| <= ||v||_1 ~ 11, safely inside fp32 exp range.
"""
import sys

sys.path.insert(0, "/opt/trn_rl_repo")

from contextlib import ExitStack

import numpy as np

import concourse.bass as bass
import concourse.tile as tile
from concourse import bacc, mybir
from concourse.bass_utils import run_bass_kernel_spmd
from concourse.masks import make_identity

B, T, ENC_DIM = 64, 2048, 512
Q_DIM, ATTN, CH, KS, PAD = 256, 128, 32, 31, 15
N_CORES = 8
BL = B // N_CORES  # 8 batches per core

f32 = mybir.dt.float32
fp16 = mybir.dt.float16
AF = mybir.ActivationFunctionType


def build(reps: int = 1, barrier: bool = False):
    nc = bacc.Bacc("TRN2", target_bir_lowering=False, debug=False,
                   num_devices=N_CORES)

    enc_d = nc.dram_tensor("encoder_outputs", [BL, T, ENC_DIM], f32,
                           kind="ExternalInput").ap()
    dh_d = nc.dram_tensor("decoder_hidden", [BL, Q_DIM], f32,
                          kind="ExternalInput").ap()
    pa_d = nc.dram_tensor("prev_attention", [BL, T], f32,
                          kind="ExternalInput").ap()
    wq_d = nc.dram_tensor("Wq", [Q_DIM, ATTN], f32, kind="ExternalInput").ap()
    wk_d = nc.dram_tensor("Wk", [ENC_DIM, ATTN], f32, kind="ExternalInput").ap()
    cw_d = nc.dram_tensor("conv_w", [CH, 1, KS], f32, kind="ExternalInput").ap()
    cb_d = nc.dram_tensor("conv_b", [CH], f32, kind="ExternalInput").ap()
    wl_d = nc.dram_tensor("Wl", [CH, ATTN], f32, kind="ExternalInput").ap()
    v_d = nc.dram_tensor("v", [ATTN], f32, kind="ExternalInput").ap()
    out_d = nc.dram_tensor("out", [BL, T], f32, kind="ExternalOutput").ap()

    # internal DRAM scratch for the zero-padded prev_attention rows
    pa_pad_d = nc.dram_tensor("pa_pad", [BL, T + 32], f32).ap()

    with tile.TileContext(nc) as tc, ExitStack() as ctx:
        singles = ctx.enter_context(tc.tile_pool(name="singles", bufs=1))
        sb_enc = ctx.enter_context(tc.tile_pool(name="enc", bufs=4))
        sb_xt = ctx.enter_context(tc.tile_pool(name="xt", bufs=4))
        sb_tanh = ctx.enter_context(tc.tile_pool(name="tanh", bufs=3))
        sb_sm = ctx.enter_context(tc.tile_pool(name="sm", bufs=2))
        ps_xt = ctx.enter_context(tc.tile_pool(name="ps_xt", bufs=3, space="PSUM"))
        ps_o = ctx.enter_context(tc.tile_pool(name="ps_o", bufs=4, space="PSUM"))
        ps_prep = ctx.enter_context(tc.tile_pool(name="ps_prep", bufs=1, space="PSUM"))

        # ---------------- constants ----------------
        ident_f = singles.tile([128, 128], f32)
        make_identity(nc, ident_f)
        ident = singles.tile([128, 128], fp16)
        nc.vector.tensor_copy(ident, ident_f)

        wk_sb = singles.tile([128, 4, ATTN], fp16)
        nc.gpsimd.dma_start(wk_sb, wk_d.rearrange("(c k) a -> k c a", c=4))
        wq_sb = singles.tile([128, 2, ATTN], fp16)
        nc.gpsimd.dma_start(wq_sb, wq_d.rearrange("(c k) a -> k c a", c=2))
        dh_sb = singles.tile([BL, Q_DIM], fp16)
        nc.gpsimd.dma_start(dh_sb, dh_d)
        cwb_sb = singles.tile([CH, 32], fp16)
        nc.gpsimd.dma_start(cwb_sb[:, 0:KS], cw_d.rearrange("c o k -> c (o k)"))
        nc.gpsimd.dma_start(
            cwb_sb[:, KS:KS + 1],
            bass.AP(tensor=cb_d.tensor, offset=0, ap=[[1, CH], [1, 1]]))
        wl_sb = singles.tile([CH, ATTN], fp16)
        nc.gpsimd.dma_start(wl_sb, wl_d)
        v_sb = singles.tile([ATTN, 1], fp16)
        nc.gpsimd.dma_start(
            v_sb, bass.AP(tensor=v_d.tensor, offset=0, ap=[[1, ATTN], [1, 1]]))

        # ---------------- prep: queryT, WW ----------------
        # dhT [256, 8] via two transpose-matmuls of dh [8, 256]
        dhT_ps = ps_prep.tile([128, 2, BL], f32, tag="prep")
        for c in range(2):
            nc.tensor.matmul(dhT_ps[:, c, :], dh_sb[:, c * 128:(c + 1) * 128],
                             ident[0:BL, 0:BL], start=True, stop=True)
        dhT_sb = singles.tile([128, 2, BL], fp16)
        nc.vector.tensor_copy(dhT_sb, dhT_ps)

        # queryT [A, 8] = Wq.T @ dhT  (accumulate 2 chunks of q-dim)
        qt_ps = ps_prep.tile([ATTN, BL], f32, tag="prep")
        for c in range(2):
            nc.tensor.matmul(qt_ps, wq_sb[:, c, :], dhT_sb[:, c, :],
                             start=(c == 0), stop=(c == 1))
        qt_sb = singles.tile([ATTN, BL], f32)
        nc.scalar.copy(qt_sb, qt_ps)

        # WW [32, A]: rows 0..30 = sum_c conv_w[c,k] Wl[c,:], row 31 = conv_b @ Wl
        ww_ps = ps_prep.tile([32, ATTN], f32, tag="prep")
        nc.tensor.matmul(ww_ps, cwb_sb, wl_sb, start=True, stop=True)
        ww_sb = singles.tile([32, ATTN], fp16)
        nc.vector.tensor_copy(ww_sb, ww_ps)

        # ---------------- prep: shifted prev_attention windows ----------------
        pa_stage = singles.tile([BL, T + 32], f32)
        nc.vector.memset(pa_stage, 0.0)
        nc.sync.dma_start(pa_stage[:, PAD:PAD + T], pa_d)
        nc.sync.dma_start(pa_pad_d, pa_stage)

        ones_sb = singles.tile([1, T], f32)
        nc.vector.memset(ones_sb, 1.0)
        ones_d = nc.dram_tensor("ones_row", [T], f32).ap()
        nc.sync.dma_start(ones_d, ones_sb)

        # pa_sh[k, b, t] = pa_pad[b, t + k]  (k=0..30), row 31 = ones
        pa_sh = singles.tile([32, BL, T], fp16)
        nc.gpsimd.dma_start(
            pa_sh[0:KS, :, :],
            bass.AP(tensor=pa_pad_d.tensor, offset=0,
                    ap=[[1, KS], [T + 32, BL], [1, T]]))
        nc.gpsimd.dma_start(
            pa_sh[KS:KS + 1, :, :],
            bass.AP(tensor=ones_d.tensor, offset=0,
                    ap=[[0, 1], [0, BL], [1, T]]))

        # ---------------- main loop ----------------
        # supertile = 1024 tokens; partition p holds tokens 8p..8p+7 of it.
        NSUP = T // 1024  # 2 per batch row
        for rep in range(reps):
            if barrier and rep:
                nc.all_engine_barrier(sem_only=True)
            exp_sb = sb_sm.tile([BL, T], f32, tag="exp")
            for b in range(BL):
                for G in range(NSUP):
                    enc_sb = sb_enc.tile([128, 8, ENC_DIM], fp16, tag="enc")
                    nc.gpsimd.dma_start(
                        enc_sb,
                        enc_d[b, G * 1024:(G + 1) * 1024, :]
                        .rearrange("(p s) f -> p s f", p=128))
                    for h in range(2):
                        out_ps = ps_o.tile([ATTN, 512], f32, tag="o")
                        for c in range(4):
                            xt_ps = ps_xt.tile([128, 512], f32, tag="xt")
                            for q in range(4):
                                nc.tensor.matmul(
                                    xt_ps[:, q * 128:(q + 1) * 128],
                                    enc_sb[:, 4 * h + q, c * 128:(c + 1) * 128],
                                    ident, start=True, stop=True)
                            xt_sb = sb_xt.tile([128, 512], fp16, tag="xts")
                            if c == 3:
                                nc.scalar.copy(xt_sb, xt_ps)
                            else:
                                nc.vector.tensor_copy(xt_sb, xt_ps)
                            nc.tensor.matmul(out_ps, wk_sb[:, c, :], xt_sb,
                                             start=(c == 0), stop=False)
                        # loc term: pa columns in permuted token order
                        # col j=(q,p) -> token 8p + 4h + q of this supertile
                        _sl = pa_sh[:, b, G * 1024 + 4 * h:]
                        pa_slice = bass.AP(tensor=_sl.tensor, offset=_sl.offset,
                                           ap=[_sl.ap[0], [1, 4], [8, 128]])
                        nc.tensor.matmul(out_ps, ww_sb, pa_slice,
                                         start=False, stop=True)

                        tanh_sb = sb_tanh.tile([ATTN, 512], fp16, tag="tanh")
                        nc.scalar.activation(tanh_sb, out_ps, AF.Tanh,
                                             bias=qt_sb[:, b:b + 1])

                        e_ps = ps_o.tile([1, 512], f32, tag="o")
                        nc.tensor.matmul(e_ps, v_sb, tanh_sb, start=True,
                                         stop=True)
                        exp_g = sb_xt.tile([1, 512], f32, tag="expg")
                        nc.scalar.activation(exp_g, e_ps, AF.Exp)
                        # ACT cannot write at partition base b; HWDGE DMA can
                        # (sync ring: keeps it off the SWDGE enc stream).
                        nc.sync.dma_start(
                            exp_sb[b:b + 1,
                                   G * 1024 + h * 512:G * 1024 + (h + 1) * 512],
                            exp_g)

            # softmax normalization over T, batched across the 8 rows
            sums = sb_sm.tile([BL, 1], f32, tag="sums")
            nc.vector.reduce_sum(sums, exp_sb, axis=mybir.AxisListType.X)
            inv = sb_sm.tile([BL, 1], f32, tag="inv")
            nc.vector.reciprocal(inv, sums)
            o_sb = sb_sm.tile([BL, T], f32, tag="osb")
            # un-permute: natural token t = G*1024 + 8p + 4h + q reads storage
            # index G*1024 + h*512 + q*128 + p
            _e = exp_sb[:, :]
            exp_perm = bass.AP(
                tensor=_e.tensor, offset=_e.offset,
                ap=[_e.ap[0], [1024, NSUP], [1, 128], [512, 2], [128, 4]])
            nc.vector.tensor_scalar_mul(o_sb, exp_perm, inv)
            nc.sync.dma_start(out_d, o_sb)

    nc.compile()
    return nc


_cache = {}


def _get(reps: int = 1, barrier: bool = False):
    key = (reps, barrier)
    if key not in _cache:
        _cache[key] = build(reps, barrier)
    return _cache[key]


def _in_maps(inputs):
    enc = np.ascontiguousarray(np.asarray(inputs["encoder_outputs"], dtype=np.float32))
    dh = np.ascontiguousarray(np.asarray(inputs["decoder_hidden"], dtype=np.float32))
    pa = np.ascontiguousarray(np.asarray(inputs["prev_attention"], dtype=np.float32))
    rep = {k: np.ascontiguousarray(np.asarray(inputs[k], dtype=np.float32))
           for k in ("Wq", "Wk", "conv_w", "conv_b", "Wl", "v")}
    maps = []
    for i in range(N_CORES):
        s = slice(i * BL, (i + 1) * BL)
        maps.append({"encoder_outputs": enc[s], "decoder_hidden": dh[s],
                     "prev_attention": pa[s], **rep})
    return maps


def kernel(**inputs) -> np.ndarray:
    nc = _get(1)
    res = run_bass_kernel_spmd(nc, _in_maps(inputs), list(range(N_CORES)))
    return np.concatenate([res.results[i]["out"] for i in range(N_CORES)],
                          axis=0).astype(np.float32)


if __name__ == "__main__":
    rng = np.random.default_rng(0)
    ins = {
        "encoder_outputs": rng.standard_normal((B, T, ENC_DIM), dtype=np.float32),
        "decoder_hidden": rng.standard_normal((B, Q_DIM), dtype=np.float32),
        "prev_attention": rng.random((B, T), dtype=np.float32),
        "Wq": (rng.standard_normal((Q_DIM, ATTN), dtype=np.float32) / np.sqrt(Q_DIM)),
        "Wk": (rng.standard_normal((ENC_DIM, ATTN), dtype=np.float32) / np.sqrt(ENC_DIM)),
        "conv_w": (rng.standard_normal((CH, 1, KS), dtype=np.float32) / np.sqrt(KS)),
        "conv_b": np.zeros(CH, dtype=np.float32),
        "Wl": (rng.standard_normal((CH, ATTN), dtype=np.float32) / np.sqrt(CH)),
        "v": (rng.standard_normal(ATTN, dtype=np.float32) / np.sqrt(ATTN)),
    }
    out = kernel(**ins)
    print("kernel output", out.shape, out.dtype, "row sums ~1:",
          np.allclose(out.sum(axis=1), 1.0, atol=1e-3))



# revision 3
# speedup vs baseline: 1.2737x; 1.2012x over previous
"""Location-sensitive attention TRN2 Bass kernel — v2 (pipelined).

Data-parallel over batch: B=64 sharded as 8 per NeuronCore across 8 cores;
parameters replicated. Per core:

  query   = decoder_hidden @ Wq                     [8, 128]   (prep, on PE)
  keys    = encoder_outputs @ Wk                    [8, 2048, 128]
  loc     = conv1d(prev_attention) ; loc_term = loc @ Wl
  energy  = tanh(keys + query + loc_term) @ v       [8, 2048]
  out     = softmax(energy, axis=T)

Key structural ideas (v2, beyond the measured-v1 design notes below):
 * Supertile-lagged transpose pipeline: during iteration s the PE
   interleaves transpose matmuls for supertile s+1 (drained PSUM->SBUF by
   DVE/ACT/GPSIMD copies with ~4.6us of slack) with keys matmuls for
   supertile s that read SBUF tiles copied during s-1. PE never waits on a
   copy, so it keeps its 2.4 GHz p-state (HAM throttles idle PE to 1.2).
 * Energy batching: the v-dot for batch b uses lhsT = v8[:, b, :] (v in
   column b, zeros elsewhere), accumulating all 8 batches' energies into
   one [8, 512] PSUM tile per (G, h). exp then runs as 4 [8, 512]
   activations with accum_out producing the softmax row-sums for free —
   removes 64 [1,512] exp ops and 64 HWDGE row-assembly DMAs.
 * Softmax tail: activation(Copy, scale=1/sum) on ACT does the normalize
   through the un-permuting read AP; DVE only computes the reciprocal.

Measured-on-HW design notes inherited from v1:
 * enc arrives [tok, feat]; transposes are REGULAR fp16 matmuls against an
   identity (transpose-mode does not register as PE activity in the HAM
   clock monitor and throttles the whole PE stream to 1.2 GHz).
 * fp16 operands: 1 cyc/row matmul rate, ~1e-3 rel err (vs 2e-2 budget).
 * enc is loaded in 2MB chunks (1024 tokens), partition p holding 8
   consecutive tokens (8KB contiguous HBM per partition): 460 GB/s/core
   measured vs 232 GB/s for a 512B-run layout. Token order inside a group
   becomes t = 8p + 4h + q; softmax sums are order-invariant and the final
   normalization read un-permutes via its access pattern.
 * enc loads ride SWDGE (gpsimd; f32->fp16 cast in the DMA), issued two
   supertiles ahead so transposes never wait on an in-flight DMA.
 * conv+Wl+conv_b fold into one [32, 128] matrix WW (rows 0..30 =
   sum_c conv_w[c,k]*Wl[c,:], row 31 = conv_b @ Wl), applied against a
   [32, 512] shifted-window view of prev_attention (row 31 = ones) and
   accumulated into the same PSUM tile as the keys matmuls. query is the
   per-partition bias of the tanh activation. exp needs no max-subtraction:
   |energy| <= ||v||_1 ~ 11, safely inside fp32 exp range.
"""
import sys

sys.path.insert(0, "/opt/trn_rl_repo")

from contextlib import ExitStack

import numpy as np

import concourse.bass as bass
import concourse.tile as tile
from concourse import bacc, mybir
from concourse.bass_utils import run_bass_kernel_spmd
from concourse.masks import make_identity

B, T, ENC_DIM = 64, 2048, 512
Q_DIM, ATTN, CH, KS, PAD = 256, 128, 32, 31, 15
N_CORES = 8
BL = B // N_CORES  # 8 batches per core
NSUP = T // 1024   # 2 supertiles per batch row
NS = BL * NSUP     # 16 supertiles per core

f32 = mybir.dt.float32
fp16 = mybir.dt.float16
AF = mybir.ActivationFunctionType


def build(reps: int = 1, barrier: bool = False):
    nc = bacc.Bacc("TRN2", target_bir_lowering=False, debug=False,
                   num_devices=N_CORES)

    enc_d = nc.dram_tensor("encoder_outputs", [BL, T, ENC_DIM], f32,
                           kind="ExternalInput").ap()
    dh_d = nc.dram_tensor("decoder_hidden", [BL, Q_DIM], f32,
                          kind="ExternalInput").ap()
    pa_d = nc.dram_tensor("prev_attention", [BL, T], f32,
                          kind="ExternalInput").ap()
    wq_d = nc.dram_tensor("Wq", [Q_DIM, ATTN], f32, kind="ExternalInput").ap()
    wk_d = nc.dram_tensor("Wk", [ENC_DIM, ATTN], f32, kind="ExternalInput").ap()
    cw_d = nc.dram_tensor("conv_w", [CH, 1, KS], f32, kind="ExternalInput").ap()
    cb_d = nc.dram_tensor("conv_b", [CH], f32, kind="ExternalInput").ap()
    wl_d = nc.dram_tensor("Wl", [CH, ATTN], f32, kind="ExternalInput").ap()
    v_d = nc.dram_tensor("v", [ATTN], f32, kind="ExternalInput").ap()
    out_d = nc.dram_tensor("out", [BL, T], f32, kind="ExternalOutput").ap()

    # supertile s -> (batch b, supertile-in-row G)
    def sb_of(s):
        return s % BL, s // BL

    with tile.TileContext(nc) as tc, ExitStack() as ctx:
        singles = ctx.enter_context(tc.tile_pool(name="singles", bufs=1))
        sb_enc = ctx.enter_context(tc.tile_pool(name="enc", bufs=6))
        sb_xt = ctx.enter_context(tc.tile_pool(name="xt", bufs=4))
        sb_tanh = ctx.enter_context(tc.tile_pool(name="tanh", bufs=4))
        sb_sm = ctx.enter_context(tc.tile_pool(name="sm", bufs=2))
        ps_xt = ctx.enter_context(tc.tile_pool(name="ps_xt", bufs=4, space="PSUM"))
        ps_o = ctx.enter_context(tc.tile_pool(name="ps_o", bufs=2, space="PSUM"))
        ps_e = ctx.enter_context(tc.tile_pool(name="ps_e", bufs=2, space="PSUM"))

        # ---------------- constants ----------------
        ident_f = singles.tile([128, 128], f32)
        make_identity(nc, ident_f)
        ident = singles.tile([128, 128], fp16)
        nc.vector.tensor_copy(ident, ident_f)

        # Param loads ride the sync HWDGE as f32 with on-chip casts: the
        # gpsimd SWDGE queue is reserved for the enc stream (SWDGE costs
        # ~1us of descriptor-gen per DMA, which would delay enc supertile 0
        # and stretch the pipeline fill).
        wk_f = singles.tile([128, 4, ATTN], f32)
        nc.sync.dma_start(wk_f, wk_d.rearrange("(c k) a -> k c a", c=4))
        wq_f = singles.tile([128, 2, ATTN], f32)
        nc.sync.dma_start(wq_f, wq_d.rearrange("(c k) a -> k c a", c=2))
        dh_f = singles.tile([BL, Q_DIM], f32)
        nc.sync.dma_start(dh_f, dh_d)
        cwb_f = singles.tile([CH, KS], f32)
        nc.sync.dma_start(cwb_f, cw_d.rearrange("c o k -> c (o k)"))
        wl_f = singles.tile([CH, ATTN], f32)
        nc.sync.dma_start(wl_f, wl_d)
        cb_f = singles.tile([CH, 1], f32)
        nc.sync.dma_start(cb_f, bass.AP(tensor=cb_d.tensor, offset=0,
                                        ap=[[1, CH], [1, 1]]))
        v_f = singles.tile([ATTN, 1], f32)
        nc.sync.dma_start(v_f, bass.AP(tensor=v_d.tensor, offset=0,
                                       ap=[[1, ATTN], [1, 1]]))

        wk_sb = singles.tile([128, 4, ATTN], fp16)
        nc.vector.tensor_copy(wk_sb, wk_f)
        wq_sb = singles.tile([128, 2, ATTN], fp16)
        nc.vector.tensor_copy(wq_sb, wq_f)
        dh_sb = singles.tile([BL, Q_DIM], fp16)
        nc.vector.tensor_copy(dh_sb, dh_f)
        cwb_sb = singles.tile([CH, KS], fp16)
        nc.scalar.copy(cwb_sb, cwb_f)
        wl_sb = singles.tile([CH, ATTN], fp16)
        nc.scalar.copy(wl_sb, wl_f)
        cb8_sb = singles.tile([CH, BL], fp16)
        for b in range(BL):
            nc.scalar.copy(cb8_sb[:, b:b + 1], cb_f)
        vf = singles.tile([ATTN, 1], fp16)
        nc.vector.tensor_copy(vf, v_f)
        # v8[a, b, r] = v[a] * (r == b): the v-dot for batch b writes row b
        # of a shared [8, 512] PSUM energy tile (zeros elsewhere, accumulated)
        v8_sb = singles.tile([ATTN, BL, BL], fp16)
        nc.vector.memset(v8_sb, 0.0)
        for b in range(BL):
            nc.vector.tensor_copy(v8_sb[:, b, b:b + 1], vf)

        # ---------------- prep: queryT, WW ----------------
        dhT_ps = ps_o.tile([128, 2, BL], f32, tag="o")
        for c in range(2):
            nc.tensor.matmul(dhT_ps[:, c, :], dh_sb[:, c * 128:(c + 1) * 128],
                             ident[0:BL, 0:BL], start=True, stop=True)
        dhT_sb = singles.tile([128, 2, BL], fp16)
        nc.vector.tensor_copy(dhT_sb, dhT_ps)

        # qt[:, b] = Wq.T dh_b + Wl.T conv_b  (the conv bias term rides the
        # same accumulation, so the pa windows need no ones-row)
        qt_ps = ps_e.tile([ATTN, BL], f32, tag="e")
        for c in range(2):
            nc.tensor.matmul(qt_ps, wq_sb[:, c, :], dhT_sb[:, c, :],
                             start=(c == 0), stop=False)
        nc.tensor.matmul(qt_ps, wl_sb, cb8_sb, start=False, stop=True)
        qt_sb = singles.tile([ATTN, BL], f32)
        nc.scalar.copy(qt_sb, qt_ps)

        ww_ps = ps_e.tile([KS, ATTN], f32, tag="e")
        nc.tensor.matmul(ww_ps, cwb_sb, wl_sb, start=True, stop=True)
        ww_sb = singles.tile([KS, ATTN], fp16)
        nc.vector.tensor_copy(ww_sb, ww_ps)

        # ---------------- prep: shifted prev_attention windows ----------------
        # fp16 staging round trip so the shifted-window gathers can ride the
        # sync HWDGE (no cast); per-batch gathers so pa_sh[.., b=0, ..] is
        # ready before the first pa matmul needs it.
        pa_f = singles.tile([BL, T], f32)
        nc.sync.dma_start(pa_f, pa_d)
        pa16 = singles.tile([BL, T + 32], fp16)
        nc.vector.memset(pa16[:, 0:PAD], 0.0)
        nc.vector.memset(pa16[:, PAD + T:], 0.0)
        nc.vector.tensor_copy(pa16[:, PAD:PAD + T], pa_f)
        pa_pad16_d = nc.dram_tensor("pa_pad16", [BL, T + 32], fp16).ap()
        nc.sync.dma_start(pa_pad16_d, pa16)

        # pa_sh[k, b, t] = pa_pad[b, t + k]  (k=0..30)
        pa_sh = singles.tile([KS, BL, T], fp16)
        for b in range(BL):
            nc.sync.dma_start(
                pa_sh[:, b, :],
                bass.AP(tensor=pa_pad16_d.tensor, offset=b * (T + 32),
                        ap=[[1, KS], [1, T]]))

        # ---------------- main loop ----------------
        # supertile = 1024 tokens; partition p holds tokens 8p..8p+7 of it.
        # col j=(q,p) of a half -> token 8p + 4h + q of the supertile.
        # PSUM->SBUF drain engines per (half, chunk): GPSIMD cannot read
        # PSUM, so split 5 DVE / 3 ACT (ACT also runs tanh+exp).
        dve_cp = lambda o, i: nc.vector.tensor_copy(o, i)
        act_cp = lambda o, i: nc.scalar.copy(o, i)
        copy_eng = {0: [dve_cp, act_cp, dve_cp, dve_cp],
                    1: [act_cp, dve_cp, act_cp, dve_cp]}

        for rep in range(reps):
            if barrier and rep:
                nc.all_engine_barrier(sem_only=True)
            exp_sb = sb_sm.tile([BL, T], f32, tag="exp")
            parts = sb_sm.tile([BL, 4], f32, tag="parts")

            enc_t = {}    # s -> enc sbuf tile
            xt_t = {}     # (s, h) -> transposed-enc sbuf tile [128, 4, 512]
            tanh_t = {}   # (s, h) -> tanh sbuf tile
            e_ps = {}     # (G, h) -> [8, 512] psum energy tile

            def emit_dma(s):
                b, G = sb_of(s)
                for h in range(2):
                    t = sb_enc.tile([128, 4, ENC_DIM], fp16, tag="enc",
                                    name=f"enc{s}_{h}")
                    enc_t[(s, h)] = t
                    # tokens 8p + s_idx, s_idx = 4h..4h+3: per-partition 8KB
                    src_ap = bass.AP(
                        tensor=enc_d.tensor,
                        offset=(b * T + G * 1024 + 4 * h) * ENC_DIM,
                        ap=[[8 * ENC_DIM, 128], [ENC_DIM, 4], [1, ENC_DIM]])
                    nc.gpsimd.dma_start(t, src_ap)

            def emit_transposes(s, h):
                # PE transposes supertile s's half h; copies drain to SBUF
                # with a full iteration of slack before keys read them.
                xt_t[(s, h)] = sb_xt.tile([128, 4, 512], fp16, tag="xts", name=f"xt{s}_{h}")
                for c in range(4):
                    xt_ps = ps_xt.tile([128, 512], f32, tag="xt")
                    for q in range(4):
                        nc.tensor.matmul(
                            xt_ps[:, q * 128:(q + 1) * 128],
                            enc_t[(s, h)][:, q, c * 128:(c + 1) * 128],
                            ident, start=True, stop=True)
                    copy_eng[h][c](xt_t[(s, h)][:, c, :], xt_ps)
                del enc_t[(s, h)]

            def emit_keys(s, h):
                b, G = sb_of(s)
                out_ps = ps_o.tile([ATTN, 512], f32, tag="o")
                for c in range(4):
                    nc.tensor.matmul(out_ps, wk_sb[:, c, :],
                                     xt_t[(s, h)][:, c, :],
                                     start=(c == 0), stop=False)
                _sl = pa_sh[:, b, G * 1024 + 4 * h:]
                pa_slice = bass.AP(tensor=_sl.tensor, offset=_sl.offset,
                                   ap=[_sl.ap[0], [1, 4], [8, 128]])
                nc.tensor.matmul(out_ps, ww_sb, pa_slice,
                                 start=False, stop=True)
                tanh_t[(s, h)] = sb_tanh.tile([ATTN, 512], fp16, tag="tanh", name=f"tanh{s}_{h}")
                nc.scalar.activation(tanh_t[(s, h)], out_ps, AF.Tanh,
                                     bias=qt_sb[:, b:b + 1])
                del xt_t[(s, h)]

            def emit_v(s, h):
                b, G = sb_of(s)
                if (G, h) not in e_ps:
                    e_ps[(G, h)] = ps_e.tile([BL, 512], f32, tag="e", name=f"e{G}_{h}")
                nc.tensor.matmul(e_ps[(G, h)], v8_sb[:, b, :], tanh_t[(s, h)],
                                 start=(b == 0), stop=(b == BL - 1))
                del tanh_t[(s, h)]

            def emit_exp(G, h):
                nc.scalar.activation(
                    exp_sb[0:BL, G * 1024 + h * 512:G * 1024 + (h + 1) * 512],
                    e_ps[(G, h)], AF.Exp,
                    accum_out=parts[:, 2 * G + h:2 * G + h + 1])
                del e_ps[(G, h)]

            # prologue: two DMAs in flight; transpose supertile 0
            emit_dma(0)
            emit_dma(1)
            for h in range(2):
                emit_transposes(0, h)

            for s in range(NS):
                if s + 2 < NS:
                    emit_dma(s + 2)
                emit_keys(s, 0)
                if s + 1 < NS:
                    emit_transposes(s + 1, 0)
                # v-dots for supertile s-1 (tanh long done); at G boundary
                # this completes e_ps of the previous G -> emit its exps.
                if s >= 1:
                    pb, pG = sb_of(s - 1)
                    for h in range(2):
                        emit_v(s - 1, h)
                    if pb == BL - 1:
                        for h in range(2):
                            emit_exp(pG, h)
                emit_keys(s, 1)
                if s + 1 < NS:
                    emit_transposes(s + 1, 1)

            # epilogue: last supertile's v-dots + exps, softmax tail
            lb, lG = sb_of(NS - 1)
            emit_v(NS - 1, 0)
            emit_exp(lG, 0)
            emit_v(NS - 1, 1)
            emit_exp(lG, 1)

            sums = sb_sm.tile([BL, 1], f32, tag="sums")
            nc.vector.reduce_sum(sums, parts, axis=mybir.AxisListType.X)
            inv = sb_sm.tile([BL, 1], f32, tag="inv")
            nc.vector.reciprocal(inv, sums)
            # un-permute: natural token t = G*1024 + 8p + 4h + q reads storage
            # index G*1024 + h*512 + q*128 + p; normalize + store per G so
            # the out DMA overlaps the second normalize.
            _e = exp_sb[:, :]
            for G in range(NSUP):
                o_g = sb_sm.tile([BL, 1024], f32, tag=f"osb{G}",
                                 name=f"osb{G}")
                exp_perm = bass.AP(
                    tensor=_e.tensor, offset=_e.offset + G * 1024,
                    ap=[_e.ap[0], [1, 128], [512, 2], [128, 4]])
                # ACT and DVE normalize one supertile-column each, in parallel
                if G == 0:
                    nc.scalar.activation(o_g, exp_perm, AF.Copy, scale=inv)
                else:
                    nc.vector.tensor_scalar_mul(o_g, exp_perm, inv)
                nc.sync.dma_start(
                    bass.AP(tensor=out_d.tensor, offset=G * 1024,
                            ap=[[T, BL], [1, 1024]]),
                    o_g)

    nc.compile()
    return nc


_cache = {}


def _get(reps: int = 1, barrier: bool = False):
    key = (reps, barrier)
    if key not in _cache:
        _cache[key] = build(reps, barrier)
    return _cache[key]


def _in_maps(inputs):
    enc = np.ascontiguousarray(np.asarray(inputs["encoder_outputs"], dtype=np.float32))
    dh = np.ascontiguousarray(np.asarray(inputs["decoder_hidden"], dtype=np.float32))
    pa = np.ascontiguousarray(np.asarray(inputs["prev_attention"], dtype=np.float32))
    rep = {k: np.ascontiguousarray(np.asarray(inputs[k], dtype=np.float32))
           for k in ("Wq", "Wk", "conv_w", "conv_b", "Wl", "v")}
    maps = []
    for i in range(N_CORES):
        s = slice(i * BL, (i + 1) * BL)
        maps.append({"encoder_outputs": enc[s], "decoder_hidden": dh[s],
                     "prev_attention": pa[s], **rep})
    return maps


def kernel(**inputs) -> np.ndarray:
    nc = _get(1)
    res = run_bass_kernel_spmd(nc, _in_maps(inputs), list(range(N_CORES)))
    return np.concatenate([res.results[i]["out"] for i in range(N_CORES)],
                          axis=0).astype(np.float32)


if __name__ == "__main__":
    rng = np.random.default_rng(0)
    ins = {
        "encoder_outputs": rng.standard_normal((B, T, ENC_DIM), dtype=np.float32),
        "decoder_hidden": rng.standard_normal((B, Q_DIM), dtype=np.float32),
        "prev_attention": rng.random((B, T), dtype=np.float32),
        "Wq": (rng.standard_normal((Q_DIM, ATTN), dtype=np.float32) / np.sqrt(Q_DIM)),
        "Wk": (rng.standard_normal((ENC_DIM, ATTN), dtype=np.float32) / np.sqrt(ENC_DIM)),
        "conv_w": (rng.standard_normal((CH, 1, KS), dtype=np.float32) / np.sqrt(KS)),
        "conv_b": np.zeros(CH, dtype=np.float32),
        "Wl": (rng.standard_normal((CH, ATTN), dtype=np.float32) / np.sqrt(CH)),
        "v": (rng.standard_normal(ATTN, dtype=np.float32) / np.sqrt(ATTN)),
    }
    out = kernel(**ins)
    print("kernel output", out.shape, out.dtype, "row sums ~1:",
          np.allclose(out.sum(axis=1), 1.0, atol=1e-3))
